# revision 1
# baseline (speedup 1.0000x reference)
"""DGCNN forward on 8 Trainium2 cores (self-contained).

Strategy: 500 graphs (200 nodes each, block-diagonal edges) padded to 512,
sharded 64 graphs/core. Device computes the 4 GCN layers exactly in fp32:
per layer  h_next = tanh((A_norm @ h) @ W)  with dense per-graph normalized
adjacency (built on host from edge_index). Graphs run 2-to-a-tile in
partition groups {0-31, 64-95} ("lanes"). Host does the cheap tail
(per-graph top-30 sort, two small convs, MLP) in exact fp32 numpy.
"""
import os
import numpy as np

N_GRAPHS, N_PER, K_TOP, F_IN, H = 500, 200, 30, 128, 32
G_PAD = 512          # padded graph count (8 cores x 64)
G_CORE = 64          # graphs per core
NL = 2               # graphs per tile (lanes at partition 0 and 64)
NQ = G_CORE // NL    # tiles per core (32)
C1_SZ, C2_SZ = 128, 72   # node chunks per graph


def _build_adj(edge_index):
    """Dense normalized adjacency per graph, A[g, d, s], fp32 (with self loops)."""
    n = N_GRAPHS * N_PER
    src = np.concatenate([edge_index[0].astype(np.int64), np.arange(n, dtype=np.int64)])
    dst = np.concatenate([edge_index[1].astype(np.int64), np.arange(n, dtype=np.int64)])
    deg = np.bincount(dst, minlength=n).astype(np.float32)
    inv = (1.0 / np.sqrt(np.maximum(deg, 1e-12))).astype(np.float32)
    w = (inv[src] * inv[dst]).astype(np.float32)
    A = np.zeros((N_GRAPHS, N_PER, N_PER), np.float32)
    np.add.at(A, (dst // N_PER, dst % N_PER, src % N_PER), w)
    return A


def _host_tail(hcat, inputs):
    """hcat [G, 200, 97] -> output [G, 1], exact fp32 numpy mirror of reference."""
    G = hcat.shape[0]
    order = np.argsort(-hcat[:, :, -1], axis=1, kind="stable")[:, :K_TOP]
    topk = np.take_along_axis(hcat, order[:, :, None], axis=1)      # [G,30,97]
    C1 = np.asarray(inputs["cw1"], np.float32)[:, 0, :].T            # [97,16]
    c1 = np.maximum(np.einsum("gkc,co->gko", topk, C1) + np.asarray(inputs["cb1"], np.float32), 0)
    p1 = np.maximum(c1[:, 0::2, :], c1[:, 1::2, :])                  # [G,15,16]
    cw2 = np.asarray(inputs["cw2"], np.float32)                      # [32,16,5]
    c2 = np.zeros((G, 11, 32), np.float32)
    for k in range(5):
        c2 += np.einsum("gti,io->gto", p1[:, k:k + 11, :], cw2[:, :, k].T)
    c2 = np.maximum(c2 + np.asarray(inputs["cb2"], np.float32), 0)
    flat = c2.transpose(0, 2, 1).reshape(G, -1)                      # [G,352]
    z = np.maximum(flat @ np.asarray(inputs["lw1"], np.float32) + np.asarray(inputs["lb1"], np.float32), 0)
    o = z @ np.asarray(inputs["lw2"], np.float32) + np.asarray(inputs["lb2"], np.float32)
    return (1.0 / (1.0 + np.exp(-o))).astype(np.float32)


def _device_gcn(xq, atq, w1, wrep2, wrep3, wrep4):
    """Run the 4 GCN layers on 8 NeuronCores. Returns oh[l] [8,NQ,128,200]."""
    import concourse.bacc as bacc
    import concourse.mybir as mybir
    import concourse.tile as tile
    from concourse import bass_utils

    dt = mybir.dt
    ACT = mybir.ActivationFunctionType
    nc = bacc.Bacc("TRN2", target_bir_lowering=False, debug=False, num_devices=8)

    d_x = nc.dram_tensor("xq", (NQ, 128, NL, 2, 128), dt.float32, kind="ExternalInput").ap()
    d_at = nc.dram_tensor("atq", (NQ, 128, NL, 2, N_PER), dt.float32, kind="ExternalInput").ap()
    d_w1 = nc.dram_tensor("w1", (128, 32), dt.float32, kind="ExternalInput").ap()
    d_w2 = nc.dram_tensor("wrep2", (128, 32), dt.float32, kind="ExternalInput").ap()
    d_w3 = nc.dram_tensor("wrep3", (128, 32), dt.float32, kind="ExternalInput").ap()
    d_w4 = nc.dram_tensor("wrep4", (128, 32), dt.float32, kind="ExternalInput").ap()
    d_oh = [nc.dram_tensor(f"oh{l}", (NQ, 128, N_PER), dt.float32, kind="ExternalOutput").ap()
            for l in range(4)]

    CSZ = (C1_SZ, C2_SZ)
    with tile.TileContext(nc) as tc:
        with tc.tile_pool(name="wp", bufs=1) as wp, \
             tc.tile_pool(name="sb", bufs=2) as sb, \
             tc.tile_pool(name="ps", bufs=2, space="PSUM") as ps:
            w1s = wp.tile([128, 32], dt.float32, name="w1s")
            nc.sync.dma_start(out=w1s[:], in_=d_w1[:])
            wls = [w1s]
            for l, dw in ((2, d_w2), (3, d_w3), (4, d_w4)):
                wt = wp.tile([128, 32], dt.float32, name=f"w{l}s")
                nc.sync.dma_start(out=wt[:], in_=dw[:])
                wls.append(wt)

            for t in range(NQ):
                xt = sb.tile([128, NL, 2, 128], dt.float32, tag="xt", name="xt")
                nc.sync.dma_start(out=xt[:], in_=d_x[t])
                at = sb.tile([128, NL, 2, N_PER], dt.float32, tag="at", name="at")
                nc.sync.dma_start(out=at[:], in_=d_at[t])

                h_prev = None
                for l in range(4):
                    # --- transform: t_l = h_prev @ W_l  (node-major psum) ---
                    tp = [ps.tile([128, NL * 32], dt.float32, tag=f"tp{c}", name=f"tp{c}")
                          for c in range(2)]
                    for c in range(2):
                        cn = CSZ[c]
                        for q in range(NL):
                            if l == 0:
                                lhsT = xt[:, q, c, 0:cn]           # [128, cn] K=128
                                rhs = w1s[:]
                            else:
                                lhsT = h_prev[64 * q:64 * q + 32, c * 128:c * 128 + cn]
                                rhs = wls[l][64 * q:64 * q + 32, :]
                            nc.tensor.matmul(tp[c][0:cn, 32 * q:32 * q + 32],
                                             lhsT=lhsT, rhs=rhs, start=True, stop=True)
                    tsb = sb.tile([128, 2, NL, 32], dt.float32, tag="tsb", name="tsb")
                    for c in range(2):
                        cn = CSZ[c]
                        nc.vector.tensor_copy(tsb[0:cn, c, :, :], tp[c][0:cn, :])
                    # --- prop: p = A @ t  (feat-major psum, lanes at rows 0/64) ---
                    pp = ps.tile([128, N_PER], dt.float32, tag="pp", name="pp")
                    for q in range(NL):
                        for c in range(2):
                            cn = CSZ[c]
                            nc.tensor.matmul(pp[64 * q:64 * q + 32, :],
                                             lhsT=tsb[0:cn, c, q, :],
                                             rhs=at[0:cn, q, c, :],
                                             start=(c == 0), stop=(c == 1))
                    # --- tanh -> h_l (feat-major sbuf), DMA out ---
                    hl = sb.tile([128, N_PER], dt.float32, tag=f"h{l}", name=f"h{l}")
                    for q in range(NL):
                        nc.scalar.activation(hl[64 * q:64 * q + 32, :],
                                             pp[64 * q:64 * q + 32, :], ACT.Tanh)
                    nc.sync.dma_start(out=d_oh[l][t], in_=hl[:])
                    h_prev = hl

    nc.compile()

    in_maps = [{"xq": xq[c], "atq": atq[c], "w1": w1,
                "wrep2": wrep2, "wrep3": wrep3, "wrep4": wrep4} for c in range(8)]
    trace = bool(int(os.environ.get("BASS_KERNEL_TRACE", "0")))
    if trace:
        try:
            import trace_hook
            trace_hook.install()
        except Exception:
            pass
    res = bass_utils.run_bass_kernel_spmd(nc, in_maps, core_ids=list(range(8)), trace=trace)
    if trace and res.exec_time_ns is not None:
        print(f"HW exec time: {res.exec_time_ns} ns")
    return [np.stack([res.results[c][f"oh{l}"] for c in range(8)]) for l in range(4)]


def kernel(**inputs):
    x = np.asarray(inputs["x"], np.float32)            # [100000, 128]
    ei = np.asarray(inputs["edge_index"])
    A = _build_adj(ei)                                  # [500, 200, 200]
    Ws = [np.asarray(inputs[f"W{i}"], np.float32) for i in (1, 2, 3, 4)]
    bs = [np.asarray(inputs[f"b{i}"], np.float32) for i in (1, 2, 3, 4)]
    xg = x.reshape(N_GRAPHS, N_PER, F_IN)

    use_device = all(np.all(b == 0) for b in bs)
    hcat = None
    if use_device:
        try:
            # ---- host prep: pad + lane layouts ----
            Ap = np.zeros((G_PAD, N_PER, N_PER), np.float32)
            Ap[:N_GRAPHS] = A
            xp = np.zeros((G_PAD, N_PER, F_IN), np.float32)
            xp[:N_GRAPHS] = xg
            xpad = np.zeros((G_PAD, 2, 128, F_IN), np.float32)
            xpad[:, 0] = xp[:, 0:128]
            xpad[:, 1, 0:C2_SZ] = xp[:, 128:200]
            # xq[core, tile, f, lane, chunk, n]
            xq = (xpad.reshape(8, NQ, NL, 2, 128, F_IN)
                      .transpose(0, 1, 5, 2, 3, 4).copy())
            AT = Ap.transpose(0, 2, 1)                            # [G, s, d]
            atp = np.zeros((G_PAD, 2, 128, N_PER), np.float32)
            atp[:, 0] = AT[:, 0:128]
            atp[:, 1, 0:C2_SZ] = AT[:, 128:200]
            # atq[core, tile, s, lane, chunk, d]
            atq = (atp.reshape(8, NQ, NL, 2, 128, N_PER)
                       .transpose(0, 1, 4, 2, 3, 5).copy())
            w1 = Ws[0]                                            # [128, 32]
            wrep = []
            for l in (1, 2, 3):
                W = Ws[l]
                Wb = W if W.shape[1] == 32 else np.tile(W, (1, 32))
                r = np.zeros((128, 32), np.float32)
                for q in range(NL):
                    r[64 * q:64 * q + 32, :] = Wb
                wrep.append(r)
            oh = _device_gcn(xq, atq, w1, wrep[0], wrep[1], wrep[2])
            # unpack: oh[l] [8, NQ, 128, 200]; graph lane q feats at rows 64q:64q+32
            hs = []
            for l in range(4):
                v = oh[l]                                          # [8,NQ,128,200]
                lanes = np.stack([v[:, :, 0:32, :], v[:, :, 64:96, :]], axis=2)
                v = lanes.transpose(0, 1, 2, 4, 3).reshape(G_PAD, N_PER, 32)
                hs.append(v[:N_GRAPHS, :, :1] if l == 3 else v[:N_GRAPHS])
            hcat = np.concatenate(hs, axis=-1)                     # [500, 200, 97]
        except Exception as e:
            print("device path failed, falling back to host:", repr(e))
            hcat = None
    if hcat is None:
        h = xg
        hs = []
        for l in range(4):
            h = np.tanh(np.einsum("gds,gsf->gdf", A, h) @ Ws[l] + bs[l])
            hs.append(h)
        hcat = np.concatenate(hs, axis=-1)
    return _host_tail(hcat, inputs)



# revision 5
# speedup vs baseline: 1.1830x; 1.1830x over previous
"""DGCNN forward on 8 Trainium2 cores (self-contained).

Strategy: 500 graphs (200 nodes, block-diagonal edges) padded to 512, 64
graphs/core, processed in 16 groups of 4 graphs.  All four GCN layers run on
device at fp32-equivalent precision using bf16 hi/lo pair arithmetic:

  transform  t = h @ W    as 2 matmuls with K-stacked lhsT [h_hi; h_lo] and
                          rhs [W_hi; W_lo] then the swapped pair -> all 4
                          cross terms accumulate in psum (exact to ~2^-17).
  propagate  p = A_n @ t  feat-major: lhsT = [t_hi | t_lo] (64 wide), stream
                          A^T_hi then A^T_lo; psum rows 0-31 + 32-63 are the
                          hi/lo partial sums, added with a shift-copy + DVE
                          add.  h = tanh(p) on the scalar engine.

Graphs sit two per psum/staging tile at partition bases 0 and 64 so every
copy/cast/sub has equal input partition bases (hardware requirement).
Host does the cheap tail (top-30 sort, convs, MLP) in exact fp32 numpy.
"""
import os
import numpy as np
import ml_dtypes

N_GRAPHS, N_PER, K_TOP, F_IN, H = 500, 200, 30, 128, 32
G_PAD = 512
G_CORE = 64          # graphs per core
NGRP = 16            # groups of 4 graphs per core
BF16 = ml_dtypes.bfloat16
C0, C1 = 128, 72     # src-node chunks (128 + 72 = 200)


def _build_adj(edge_index):
    """Dense normalized adjacency per graph, A[g, d, s], fp32 (with self loops)."""
    n = N_GRAPHS * N_PER
    src = np.concatenate([edge_index[0].astype(np.int64), np.arange(n, dtype=np.int64)])
    dst = np.concatenate([edge_index[1].astype(np.int64), np.arange(n, dtype=np.int64)])
    deg = np.bincount(dst, minlength=n).astype(np.float32)
    inv = (1.0 / np.sqrt(np.maximum(deg, 1e-12))).astype(np.float32)
    w = (inv[src] * inv[dst]).astype(np.float32)
    A = np.zeros((N_GRAPHS, N_PER, N_PER), np.float32)
    np.add.at(A, (dst // N_PER, dst % N_PER, src % N_PER), w)
    return A


def _host_tail(hcat, inputs):
    """hcat [G, 200, 97] -> output [G, 1], exact fp32 numpy mirror of reference."""
    G = hcat.shape[0]
    order = np.argsort(-hcat[:, :, -1], axis=1, kind="stable")[:, :K_TOP]
    topk = np.take_along_axis(hcat, order[:, :, None], axis=1)      # [G,30,97]
    C1w = np.asarray(inputs["cw1"], np.float32)[:, 0, :].T           # [97,16]
    c1 = np.maximum(np.einsum("gkc,co->gko", topk, C1w) + np.asarray(inputs["cb1"], np.float32), 0)
    p1 = np.maximum(c1[:, 0::2, :], c1[:, 1::2, :])                  # [G,15,16]
    cw2 = np.asarray(inputs["cw2"], np.float32)                      # [32,16,5]
    c2 = np.zeros((G, 11, 32), np.float32)
    for k in range(5):
        c2 += np.einsum("gti,io->gto", p1[:, k:k + 11, :], cw2[:, :, k].T)
    c2 = np.maximum(c2 + np.asarray(inputs["cb2"], np.float32), 0)
    flat = c2.transpose(0, 2, 1).reshape(G, -1)                      # [G,352]
    z = np.maximum(flat @ np.asarray(inputs["lw1"], np.float32) + np.asarray(inputs["lb1"], np.float32), 0)
    o = z @ np.asarray(inputs["lw2"], np.float32) + np.asarray(inputs["lb2"], np.float32)
    return (1.0 / (1.0 + np.exp(-o))).astype(np.float32)


def _split(a):
    hi = a.astype(BF16)
    lo = (a - hi.astype(np.float32)).astype(BF16)
    return hi, lo


def _device_gcn(ins):
    """Run the 4 GCN layers on 8 NeuronCores.

    ins: per-core dict arrays (see kernel() prep).  Returns list of 8 result
    dicts with H (layers 1-3) and H4.
    """
    import concourse.bacc as bacc
    import concourse.mybir as mybir
    import concourse.tile as tile
    from concourse import bass_utils

    dt = mybir.dt
    ACT = mybir.ActivationFunctionType
    OP = mybir.AluOpType
    nc = bacc.Bacc("TRN2", target_bir_lowering=False, debug=False, num_devices=8)

    d = {}
    for name, shape, ddt in [
        ("ahi0", (NGRP, 128, 800), dt.bfloat16), ("alo0", (NGRP, 128, 800), dt.bfloat16),
        ("ahi1", (NGRP, 72, 800), dt.bfloat16), ("alo1", (NGRP, 72, 800), dt.bfloat16),
        ("xA", (NGRP, 128, 800), dt.bfloat16), ("xB", (NGRP, 128, 800), dt.bfloat16),
        ("w1a", (128, 32), dt.bfloat16), ("w1as", (128, 32), dt.bfloat16),
        ("w1b", (128, 32), dt.bfloat16), ("w1bs", (128, 32), dt.bfloat16),
        ("w2r", (128, 32), dt.bfloat16), ("w2s", (128, 32), dt.bfloat16),
        ("w3r", (128, 32), dt.bfloat16), ("w3s", (128, 32), dt.bfloat16),
        ("w4r", (128, 1), dt.bfloat16), ("w4s", (128, 1), dt.bfloat16),
    ]:
        d[name] = nc.dram_tensor(name, shape, ddt, kind="ExternalInput").ap()
    # out: H [grp, layer(0..2), pair, half, 32, 200]; H4 [grp, pair, parity, 200]
    d_H = nc.dram_tensor("H", (NGRP, 3, 2, 2, 32, 200), dt.float32, kind="ExternalOutput").ap()
    # H4: pre-tanh hi/lo psum rows per graph; host does tanh(hi + lo)
    d_H4 = nc.dram_tensor("H4", (NGRP, 2, 2, 2, 200), dt.float32, kind="ExternalOutput").ap()

    with tile.TileContext(nc) as tc:
        with tc.tile_pool(name="wp", bufs=1) as wp, \
             tc.tile_pool(name="ain", bufs=2) as ain, \
             tc.tile_pool(name="sb", bufs=2) as sb, \
             tc.tile_pool(name="hhp", bufs=2) as hhp, \
             tc.tile_pool(name="pst", bufs=2, space="PSUM") as pst, \
             tc.tile_pool(name="psp", bufs=2, space="PSUM") as psp:
            W = {}
            for name in ["w1a", "w1as", "w1b", "w1bs", "w2r", "w2s", "w3r", "w3s"]:
                W[name] = wp.tile([128, 32], dt.bfloat16, name=name)
                nc.sync.dma_start(out=W[name][:], in_=d[name])
            for name in ["w4r", "w4s"]:
                W[name] = wp.tile([128, 1], dt.bfloat16, name=name)
                nc.sync.dma_start(out=W[name][:], in_=d[name])
            WR = [(W["w1a"], W["w1as"], W["w1b"], W["w1bs"]),
                  (W["w2r"], W["w2s"]), (W["w3r"], W["w3s"]), (W["w4r"], W["w4s"])]

            for grp in range(NGRP):
                A = {}
                for nm, p in [("ahi0", 128), ("alo0", 128), ("ahi1", 72), ("alo1", 72)]:
                    A[nm] = ain.tile([p, 800], dt.bfloat16, tag=nm, name=nm)
                    nc.sync.dma_start(out=A[nm][:], in_=d[nm][grp])
                xA = ain.tile([128, 800], dt.bfloat16, tag="xA", name="xA")
                nc.sync.dma_start(out=xA[:], in_=d["xA"][grp])
                xB = ain.tile([128, 800], dt.bfloat16, tag="xB", name="xB")
                nc.sync.dma_start(out=xB[:], in_=d["xB"][grp])

                hh_prev = [None, None]   # per pair: [128,200] = [g0hi;g0lo;g1hi;g1lo]
                for l in range(4):
                    tw = 1 if l == 3 else 32    # transform output width
                    for pr in range(2):
                        # ---------- transforms + t-splits + props ----------
                        p2 = psp.tile([128, 200], dt.float32, tag=f"p2{pr}", name=f"p2{pr}")
                        for par in range(2):     # graph parity within pair
                            g = 2 * pr + par
                            gc = slice(200 * g, 200 * g + 200)
                            t2 = pst.tile([128, 2 * tw], dt.float32, tag=f"t2{par}", name=f"t2{par}")
                            for c, (cn, nsl) in enumerate(((C0, slice(200 * g, 200 * g + 128)),
                                                           (C1, slice(200 * g + 128, 200 * g + 200)))):
                                out = t2[0:cn, c * tw:(c + 1) * tw]
                                if l == 0:
                                    mms = [(xA[:, nsl], W["w1a"]), (xA[:, nsl], W["w1as"]),
                                           (xB[:, nsl], W["w1b"]), (xB[:, nsl], W["w1bs"])]
                                else:
                                    hh = hh_prev[pr]
                                    rsl = slice(64 * par, 64 * par + 64)
                                    lsl = slice(128 * par + c * 128, 128 * par + c * 128 + cn)
                                    # hh cols: [0:200] per graph; chunk c cols
                                    hsl = hh[64 * par:64 * par + 64, c * 128:c * 128 + cn]
                                    wr = WR[l]
                                    mms = [(hsl, wr[0][rsl, 0:tw]), (hsl, wr[1][rsl, 0:tw])]
                                for i, (lh, rh) in enumerate(mms):
                                    nc.tensor.matmul(out, lhsT=lh, rhs=rh,
                                                     start=(i == 0), stop=(i == len(mms) - 1))
                            # split t -> tl [128, 4*tw] bf16: [c0hi|c0lo|c1hi|c1lo]
                            tl = sb.tile([128, 4 * tw], dt.bfloat16, tag=f"tl{par}", name=f"tl{par}")
                            nc.vector.tensor_copy(tl[:, 0:tw], t2[:, 0:tw])
                            nc.vector.tensor_tensor(tl[:, tw:2 * tw], t2[:, 0:tw], tl[:, 0:tw], OP.subtract)
                            nc.vector.tensor_copy(tl[0:72, 2 * tw:3 * tw], t2[0:72, tw:2 * tw])
                            nc.vector.tensor_tensor(tl[0:72, 3 * tw:4 * tw], t2[0:72, tw:2 * tw],
                                                    tl[0:72, 2 * tw:3 * tw], OP.subtract)
                            # propagate: psum rows 64*par..(+2*tw): hi block + lo block
                            pout = p2[64 * par:64 * par + 2 * tw, :]
                            nc.tensor.matmul(pout, lhsT=tl[:, 0:2 * tw], rhs=A["ahi0"][:, gc],
                                             start=True, stop=False)
                            nc.tensor.matmul(pout, lhsT=tl[:, 0:2 * tw], rhs=A["alo0"][:, gc],
                                             start=False, stop=False)
                            nc.tensor.matmul(pout, lhsT=tl[0:72, 2 * tw:4 * tw], rhs=A["ahi1"][0:72, gc],
                                             start=False, stop=False)
                            nc.tensor.matmul(pout, lhsT=tl[0:72, 2 * tw:4 * tw], rhs=A["alo1"][0:72, gc],
                                             start=False, stop=True)
                        if l == 3:
                            # ship pre-tanh hi/lo rows; host adds + tanh
                            ps4 = sb.tile([128, 200], dt.float32, tag=f"ps4{pr}", name=f"ps4{pr}")
                            nc.vector.tensor_copy(ps4[0:2, :], p2[0:2, :])
                            nc.vector.tensor_copy(ps4[64:66, :], p2[64:66, :])
                            nc.sync.dma_start(out=d_H4[grp, pr, 0], in_=ps4[0:2, :])
                            nc.sync.dma_start(out=d_H4[grp, pr, 1], in_=ps4[64:66, :])
                            continue
                        # ---------- pair-sum + tanh ----------
                        tmp = sb.tile([128, 200], dt.float32, tag=f"tmp{pr}", name=f"tmp{pr}")
                        P = sb.tile([128, 200], dt.float32, tag=f"P{pr}", name=f"P{pr}")
                        for par in range(2):
                            b = 64 * par
                            nc.vector.tensor_copy(tmp[b:b + tw, :], p2[b + tw:b + 2 * tw, :])
                            nc.vector.tensor_tensor(P[b:b + tw, :], p2[b:b + tw, :],
                                                    tmp[b:b + tw, :], OP.add)
                        Ht = sb.tile([128, 200], dt.float32, tag=f"H{pr}", name=f"H{pr}")
                        nc.scalar.activation(Ht[:], P[:], ACT.Tanh)
                        nc.sync.dma_start(out=d_H[grp, l, pr, 0], in_=Ht[0:32, :])
                        nc.sync.dma_start(out=d_H[grp, l, pr, 1], in_=Ht[64:96, :])
                        # split h -> hh [128,200] = [g0hi;g0lo;g1hi;g1lo]
                        hh = hhp.tile([128, 200], dt.bfloat16, tag=f"hh{pr}", name=f"hh{pr}")
                        nc.scalar.activation(hh[0:32, :], Ht[0:32, :], ACT.Copy)
                        nc.vector.tensor_tensor(hh[32:64, :], Ht[0:32, :], hh[0:32, :], OP.subtract)
                        nc.scalar.activation(hh[64:96, :], Ht[64:96, :], ACT.Copy)
                        nc.vector.tensor_tensor(hh[96:128, :], Ht[64:96, :], hh[64:96, :], OP.subtract)
                        hh_prev[pr] = hh

    nc.compile()

    trace = bool(int(os.environ.get("BASS_KERNEL_TRACE", "0")))
    res = bass_utils.run_bass_kernel_spmd(nc, ins, core_ids=list(range(8)), trace=trace)
    if trace and res.exec_time_ns is not None:
        print(f"HW exec time: {res.exec_time_ns} ns")
    return res.results


def kernel(**inputs):
    x = np.asarray(inputs["x"], np.float32)            # [100000, 128]
    ei = np.asarray(inputs["edge_index"])
    A = _build_adj(ei)                                  # [500, 200, 200]
    Ws = [np.asarray(inputs[f"W{i}"], np.float32) for i in (1, 2, 3, 4)]
    bs = [np.asarray(inputs[f"b{i}"], np.float32) for i in (1, 2, 3, 4)]
    xg = x.reshape(N_GRAPHS, N_PER, F_IN)

    use_device = all(np.all(b == 0) for b in bs)
    hcat = None
    if use_device:
        try:
            # ---- host prep ----
            At = np.zeros((G_PAD, N_PER, N_PER), np.float32)
            At[:N_GRAPHS] = A.transpose(0, 2, 1)        # [g, src, dst]
            Ahi, Alo = _split(At)
            xt = np.zeros((G_PAD, F_IN, N_PER), np.float32)
            xt[:N_GRAPHS] = xg.transpose(0, 2, 1)       # [g, feat, node]
            xhi, xlo = _split(xt)
            xAa = np.concatenate([xhi[:, 0:64], xlo[:, 0:64]], axis=1)      # [512,128,200]
            xBb = np.concatenate([xhi[:, 64:128], xlo[:, 64:128]], axis=1)

            def core_view(arr, rows):
                # arr [512, rows, 200] -> [8, NGRP, rows, 4*200]
                return (arr.reshape(8, NGRP, 4, rows, 200)
                           .transpose(0, 1, 3, 2, 4).reshape(8, NGRP, rows, 800).copy())

            ahi0 = core_view(np.ascontiguousarray(Ahi[:, 0:128]).astype(BF16), 128)
            alo0 = core_view(np.ascontiguousarray(Alo[:, 0:128]).astype(BF16), 128)
            ahi1 = core_view(np.ascontiguousarray(Ahi[:, 128:200]).astype(BF16), 72)
            alo1 = core_view(np.ascontiguousarray(Alo[:, 128:200]).astype(BF16), 72)
            xA = core_view(xAa, 128)
            xB = core_view(xBb, 128)

            def wpair(Wm, rep):
                hi, lo = _split(Wm)
                pair = np.concatenate([hi.astype(np.float32), lo.astype(np.float32)], axis=0)
                swap = np.concatenate([lo.astype(np.float32), hi.astype(np.float32)], axis=0)
                if rep:
                    pair = np.concatenate([pair, pair], axis=0)
                    swap = np.concatenate([swap, swap], axis=0)
                return pair.astype(BF16), swap.astype(BF16)

            w1a, w1as = wpair(Ws[0][0:64], False)       # [128, 32]
            w1b, w1bs = wpair(Ws[0][64:128], False)
            w2r, w2s = wpair(Ws[1], True)
            w3r, w3s = wpair(Ws[2], True)
            w4r, w4s = wpair(Ws[3], True)               # [128, 1]

            ins = [{"ahi0": ahi0[c], "alo0": alo0[c], "ahi1": ahi1[c], "alo1": alo1[c],
                    "xA": xA[c], "xB": xB[c],
                    "w1a": w1a, "w1as": w1as, "w1b": w1b, "w1bs": w1bs,
                    "w2r": w2r, "w2s": w2s, "w3r": w3r, "w3s": w3s,
                    "w4r": w4r, "w4s": w4s} for c in range(8)]
            res = _device_gcn(ins)

            # ---- unpack: H [NGRP,3,2,2,32,200], H4 [NGRP,2,2,200] per core ----
            hs = []
            for l in range(3):
                v = np.stack([res[c]["H"][:, l] for c in range(8)])   # [8,16,2,2,32,200]
                v = v.reshape(8, NGRP, 4, 32, 200).transpose(0, 1, 2, 4, 3)
                hs.append(v.reshape(G_PAD, N_PER, 32)[:N_GRAPHS])
            v4 = np.stack([res[c]["H4"] for c in range(8)])           # [8,16,2,2,2,200]
            h4 = np.tanh(v4[..., 0, :] + v4[..., 1, :])               # [8,16,2,2,200]
            h4 = h4.reshape(G_PAD, N_PER, 1)[:N_GRAPHS]
            hcat = np.concatenate(hs + [h4], axis=-1)                 # [500,200,97]
        except Exception as e:
            print("device path failed, falling back to host:", repr(e))
            hcat = None
    if hcat is None:
        h = xg
        hs = []
        for l in range(4):
            h = np.tanh(np.einsum("gds,gsf->gdf", A, h) @ Ws[l] + bs[l])
            hs.append(h)
        hcat = np.concatenate(hs, axis=-1)
    return _host_tail(hcat, inputs)


# revision 10
# speedup vs baseline: 1.5693x; 1.3265x over previous
"""DGCNN forward on 8 Trainium2 cores (self-contained).

500 graphs (200 nodes, block-diag edges) padded to 512, 64 graphs/core in 16
groups of 4.  All four GCN layers run on device at fp32-equivalent precision
via bf16 hi/lo pair arithmetic:

  transform  t = h @ W   2 matmuls: K-stacked lhsT [h_hi; h_lo] with rhs
                         [W_hi; W_lo] then swapped -> all 4 cross terms
                         accumulate in psum (exact to ~2^-17).
  propagate  p = A_n @ t feat-major: lhsT = [t_hi | t_lo] (64 wide, strided
                         AP), stream A^T_hi then A^T_lo per 128/72 src chunk;
                         psum rows 0-31/32-63 hold hi/lo partials, summed by
                         shift-copy + DVE add.  h = tanh(p) on scalar.

Two graphs per psum/staging tile at partition bases 0/64 keep all engine op
partition bases equal (hw requirement).  Groups are processed layer-major in
two sets of 8 so independent per-group chains interleave in the FIFO engine
queues.  Host does the cheap tail (top-30 sort, convs, MLP) in fp32 numpy.
"""
import os
import numpy as np
import ml_dtypes

N_GRAPHS, N_PER, K_TOP, F_IN, H = 500, 200, 30, 128, 32
G_PAD = 512
G_CORE = 64
NGRP = 16
NSET = 8            # groups per resident set
BF16 = ml_dtypes.bfloat16
C0, C1 = 128, 72


def _build_adj(edge_index):
    n = N_GRAPHS * N_PER
    src = np.concatenate([edge_index[0].astype(np.int64), np.arange(n, dtype=np.int64)])
    dst = np.concatenate([edge_index[1].astype(np.int64), np.arange(n, dtype=np.int64)])
    deg = np.bincount(dst, minlength=n).astype(np.float32)
    inv = (1.0 / np.sqrt(np.maximum(deg, 1e-12))).astype(np.float32)
    w = (inv[src] * inv[dst]).astype(np.float32)
    A = np.zeros((N_GRAPHS, N_PER, N_PER), np.float32)
    np.add.at(A, (dst // N_PER, dst % N_PER, src % N_PER), w)
    return A


def _host_tail(hcat, inputs):
    G = hcat.shape[0]
    order = np.argsort(-hcat[:, :, -1], axis=1, kind="stable")[:, :K_TOP]
    topk = np.take_along_axis(hcat, order[:, :, None], axis=1)
    C1w = np.asarray(inputs["cw1"], np.float32)[:, 0, :].T
    c1 = np.maximum(np.einsum("gkc,co->gko", topk, C1w) + np.asarray(inputs["cb1"], np.float32), 0)
    p1 = np.maximum(c1[:, 0::2, :], c1[:, 1::2, :])
    cw2 = np.asarray(inputs["cw2"], np.float32)
    c2 = np.zeros((G, 11, 32), np.float32)
    for k in range(5):
        c2 += np.einsum("gti,io->gto", p1[:, k:k + 11, :], cw2[:, :, k].T)
    c2 = np.maximum(c2 + np.asarray(inputs["cb2"], np.float32), 0)
    flat = c2.transpose(0, 2, 1).reshape(G, -1)
    z = np.maximum(flat @ np.asarray(inputs["lw1"], np.float32) + np.asarray(inputs["lb1"], np.float32), 0)
    o = z @ np.asarray(inputs["lw2"], np.float32) + np.asarray(inputs["lb2"], np.float32)
    return (1.0 / (1.0 + np.exp(-o))).astype(np.float32)


def _split(a):
    hi = a.astype(BF16)
    lo = (a - hi.astype(np.float32)).astype(BF16)
    return hi, lo


def _device_gcn(ins):
    import concourse.bacc as bacc
    import concourse.mybir as mybir
    import concourse.tile as tile
    from concourse import bass_utils

    dt = mybir.dt
    ACT = mybir.ActivationFunctionType
    OP = mybir.AluOpType
    nc = bacc.Bacc("TRN2", target_bir_lowering=False, debug=False, num_devices=8)

    d = {}
    for name, shape, ddt in [
        ("aX", (NGRP, 128, 1600), dt.bfloat16),   # [ahi0 | alo0]
        ("aY", (NGRP, 72, 1600), dt.bfloat16),    # [ahi1 | alo1]
        ("xAB", (NGRP, 128, 1600), dt.bfloat16),  # [xA | xB]
        ("w1a", (128, 32), dt.bfloat16), ("w1as", (128, 32), dt.bfloat16),
        ("w1b", (128, 32), dt.bfloat16), ("w1bs", (128, 32), dt.bfloat16),
        ("w2r", (128, 32), dt.bfloat16), ("w2s", (128, 32), dt.bfloat16),
        ("w3r", (128, 32), dt.bfloat16), ("w3s", (128, 32), dt.bfloat16),
        ("w4r", (128, 1), dt.bfloat16), ("w4s", (128, 1), dt.bfloat16),
    ]:
        d[name] = nc.dram_tensor(name, shape, ddt, kind="ExternalInput").ap()
    d_H = nc.dram_tensor("H", (NGRP, 3, 2, 128, 200), dt.float32, kind="ExternalOutput").ap()
    d_H4 = nc.dram_tensor("H4", (NGRP, 2, 2, 2, 200), dt.float32, kind="ExternalOutput").ap()

    with tile.TileContext(nc) as tc:
        with tc.tile_pool(name="wp", bufs=1) as wp, \
             tc.tile_pool(name="ain", bufs=2) as ain, \
             tc.tile_pool(name="sb", bufs=3) as sb, \
             tc.tile_pool(name="hhp", bufs=2) as hhp, \
             tc.tile_pool(name="pst", bufs=2, space="PSUM") as pst, \
             tc.tile_pool(name="psp", bufs=2, space="PSUM") as psp:
            W = {}
            for name in ["w1a", "w1as", "w1b", "w1bs", "w2r", "w2s", "w3r", "w3s"]:
                W[name] = wp.tile([128, 32], dt.bfloat16, name=name)
                nc.sync.dma_start(out=W[name][:], in_=d[name])
            for name in ["w4r", "w4s"]:
                W[name] = wp.tile([128, 1], dt.bfloat16, name=name)
                nc.sync.dma_start(out=W[name][:], in_=d[name])
            WR = [None, (W["w2r"], W["w2s"]), (W["w3r"], W["w3s"]), (W["w4r"], W["w4s"])]

            for st in range(NGRP // NSET):
                AX, AY, XT = {}, {}, {}
                for sl in range(NSET):
                    grp = st * NSET + sl
                    AX[sl] = ain.tile([128, 1600], dt.bfloat16, tag=f"aX{sl}", name=f"aX{sl}")
                    nc.sync.dma_start(out=AX[sl][:], in_=d["aX"][grp])
                    AY[sl] = ain.tile([72, 1600], dt.bfloat16, tag=f"aY{sl}", name=f"aY{sl}")
                    nc.sync.dma_start(out=AY[sl][:], in_=d["aY"][grp])
                    XT[sl] = ain.tile([128, 1600], dt.bfloat16, tag=f"xAB{sl}", name=f"xAB{sl}")
                    nc.sync.dma_start(out=XT[sl][:], in_=d["xAB"][grp])
                hh_prev = {}
                for l in range(4):
                    tw = 1 if l == 3 else 32
                    for sl in range(NSET):
                        grp = st * NSET + sl
                        for pr in range(2):
                            p2 = psp.tile([128, 200], dt.float32, tag=f"p2{pr}", name=f"p2{pr}")
                            for par in range(2):
                                g = 2 * pr + par
                                gc = slice(200 * g, 200 * g + 200)
                                gc2 = slice(800 + 200 * g, 800 + 200 * g + 200)
                                t2 = pst.tile([128, 2, tw], dt.float32, tag=f"t2{par}", name=f"t2{par}")
                                for c, cn in enumerate((C0, C1)):
                                    out = t2[0:cn, c, :]
                                    nsl = slice(200 * g + c * 128, 200 * g + c * 128 + cn)
                                    nsl2 = slice(800 + 200 * g + c * 128, 800 + 200 * g + c * 128 + cn)
                                    if l == 0:
                                        mms = [(XT[sl][:, nsl], W["w1a"]), (XT[sl][:, nsl], W["w1as"]),
                                               (XT[sl][:, nsl2], W["w1b"]), (XT[sl][:, nsl2], W["w1bs"])]
                                    else:
                                        hh = hh_prev[(sl, pr)]
                                        rsl = slice(64 * par, 64 * par + 64)
                                        hsl = hh[64 * par:64 * par + 64, c * 128:c * 128 + cn]
                                        wr = WR[l]
                                        mms = [(hsl, wr[0][rsl, 0:tw]), (hsl, wr[1][rsl, 0:tw])]
                                    for i, (lh, rh) in enumerate(mms):
                                        nc.tensor.matmul(out, lhsT=lh, rhs=rh,
                                                         start=(i == 0), stop=(i == len(mms) - 1))
                                # split t: tl [128, c, hi/lo, tw]
                                tl = sb.tile([128, 2, 2, tw], dt.bfloat16, tag=f"tl{par}", name=f"tl{par}")
                                nc.scalar.activation(tl[:, :, 0, :], t2[:, :, :], ACT.Copy)
                                nc.vector.tensor_tensor(tl[:, :, 1, :], t2[:, :, :], tl[:, :, 0, :], OP.subtract)
                                # propagate
                                pout = p2[64 * par:64 * par + 2 * tw, :]
                                lh0 = tl[:, 0]                  # [c0hi | c0lo] contiguous
                                lh1 = tl[0:72, 1]               # [c1hi | c1lo] contiguous
                                nc.tensor.matmul(pout, lhsT=lh0, rhs=AX[sl][:, gc],
                                                 start=True, stop=False)
                                nc.tensor.matmul(pout, lhsT=lh0, rhs=AX[sl][:, gc2],
                                                 start=False, stop=False)
                                nc.tensor.matmul(pout, lhsT=lh1, rhs=AY[sl][0:72, gc],
                                                 start=False, stop=False)
                                nc.tensor.matmul(pout, lhsT=lh1, rhs=AY[sl][0:72, gc2],
                                                 start=False, stop=True)
                            if l == 3:
                                ps4 = sb.tile([128, 200], dt.float32, tag=f"ps4{pr}", name=f"ps4{pr}")
                                nc.scalar.activation(ps4[0:2, :], p2[0:2, :], ACT.Copy)
                                nc.scalar.activation(ps4[32:34, :], p2[64:66, :], ACT.Copy)
                                nc.sync.dma_start(out=d_H4[grp, pr, 0], in_=ps4[0:2, :])
                                nc.sync.dma_start(out=d_H4[grp, pr, 1], in_=ps4[32:34, :])
                                continue
                            # pair-sum + tanh
                            tmp = sb.tile([128, 200], dt.float32, tag=f"tmp{pr}", name=f"tmp{pr}")
                            P = sb.tile([128, 200], dt.float32, tag=f"P{pr}", name=f"P{pr}")
                            for par in range(2):
                                b = 64 * par
                                nc.vector.tensor_copy(tmp[b:b + 32, :], p2[b + 32:b + 64, :])
                                nc.vector.tensor_tensor(P[b:b + 32, :], p2[b:b + 32, :],
                                                        tmp[b:b + 32, :], OP.add)
                            Ht = sb.tile([128, 200], dt.float32, tag=f"H{pr}", name=f"H{pr}")
                            nc.scalar.activation(Ht[:], P[:], ACT.Tanh)
                            nc.sync.dma_start(out=d_H[grp, l, pr], in_=Ht[:])
                            hh = hhp.tile([128, 200], dt.bfloat16, tag=f"hh{sl}{pr}", name=f"hh{sl}{pr}")
                            nc.scalar.activation(hh[0:32, :], Ht[0:32, :], ACT.Copy)
                            nc.vector.tensor_tensor(hh[32:64, :], Ht[0:32, :], hh[0:32, :], OP.subtract)
                            nc.scalar.activation(hh[64:96, :], Ht[64:96, :], ACT.Copy)
                            nc.vector.tensor_tensor(hh[96:128, :], Ht[64:96, :], hh[64:96, :], OP.subtract)
                            hh_prev[(sl, pr)] = hh

    nc.compile()

    trace = bool(int(os.environ.get("BASS_KERNEL_TRACE", "0")))
    res = bass_utils.run_bass_kernel_spmd(nc, ins, core_ids=list(range(8)), trace=trace)
    if trace and res.exec_time_ns is not None:
        print(f"HW exec time: {res.exec_time_ns} ns")
    return res.results


def kernel(**inputs):
    x = np.asarray(inputs["x"], np.float32)
    ei = np.asarray(inputs["edge_index"])
    A = _build_adj(ei)
    Ws = [np.asarray(inputs[f"W{i}"], np.float32) for i in (1, 2, 3, 4)]
    bs = [np.asarray(inputs[f"b{i}"], np.float32) for i in (1, 2, 3, 4)]
    xg = x.reshape(N_GRAPHS, N_PER, F_IN)

    use_device = all(np.all(b == 0) for b in bs)
    hcat = None
    if use_device:
        try:
            At = np.zeros((G_PAD, N_PER, N_PER), np.float32)
            At[:N_GRAPHS] = A.transpose(0, 2, 1)
            Ahi, Alo = _split(At)
            xt = np.zeros((G_PAD, F_IN, N_PER), np.float32)
            xt[:N_GRAPHS] = xg.transpose(0, 2, 1)
            xhi, xlo = _split(xt)
            xAa = np.concatenate([xhi[:, 0:64].astype(np.float32), xlo[:, 0:64].astype(np.float32)], axis=1)
            xBb = np.concatenate([xhi[:, 64:128].astype(np.float32), xlo[:, 64:128].astype(np.float32)], axis=1)

            def core_view(arr, rows):
                # arr [512, rows, 200] bf16 -> [8, NGRP, rows, 800]
                return (arr.reshape(8, NGRP, 4, rows, 200)
                           .transpose(0, 1, 3, 2, 4).reshape(8, NGRP, rows, 800).copy())

            ahi0 = core_view(np.ascontiguousarray(Ahi[:, 0:128]), 128)
            alo0 = core_view(np.ascontiguousarray(Alo[:, 0:128]), 128)
            ahi1 = core_view(np.ascontiguousarray(Ahi[:, 128:200]), 72)
            alo1 = core_view(np.ascontiguousarray(Alo[:, 128:200]), 72)
            xA = core_view(xAa.astype(BF16), 128)
            xB = core_view(xBb.astype(BF16), 128)
            aX = np.concatenate([ahi0, alo0], axis=3)      # [8, NGRP, 128, 1600]
            aY = np.concatenate([ahi1, alo1], axis=3)      # [8, NGRP, 72, 1600]
            xAB = np.concatenate([xA, xB], axis=3)

            def wpair(Wm, rep):
                hi, lo = _split(Wm)
                pair = np.concatenate([hi.astype(np.float32), lo.astype(np.float32)], axis=0)
                swap = np.concatenate([lo.astype(np.float32), hi.astype(np.float32)], axis=0)
                if rep:
                    pair = np.concatenate([pair, pair], axis=0)
                    swap = np.concatenate([swap, swap], axis=0)
                return pair.astype(BF16), swap.astype(BF16)

            w1a, w1as = wpair(Ws[0][0:64], False)
            w1b, w1bs = wpair(Ws[0][64:128], False)
            w2r, w2s = wpair(Ws[1], True)
            w3r, w3s = wpair(Ws[2], True)
            w4r, w4s = wpair(Ws[3], True)

            ins = [{"aX": aX[c], "aY": aY[c], "xAB": xAB[c],
                    "w1a": w1a, "w1as": w1as, "w1b": w1b, "w1bs": w1bs,
                    "w2r": w2r, "w2s": w2s, "w3r": w3r, "w3s": w3s,
                    "w4r": w4r, "w4s": w4s} for c in range(8)]
            res = _device_gcn(ins)

            hs = []
            for l in range(3):
                v = np.stack([res[c]["H"][:, l] for c in range(8)])   # [8,16,2,128,200]
                v = v.reshape(8, NGRP, 2, 4, 32, 200)[:, :, :, [0, 2], :, :]  # rows 0-31,64-95
                v = v.reshape(8, NGRP, 4, 32, 200).transpose(0, 1, 2, 4, 3)
                hs.append(v.reshape(G_PAD, N_PER, 32)[:N_GRAPHS])
            v4 = np.stack([res[c]["H4"] for c in range(8)])           # [8,16,2,2,2,200]
            h4 = np.tanh(v4[..., 0, :] + v4[..., 1, :])               # [8,16,2,2,200]
            h4 = h4.reshape(G_PAD, N_PER, 1)[:N_GRAPHS]
            hcat = np.concatenate(hs + [h4], axis=-1)
        except Exception as e:
            print("device path failed, falling back to host:", repr(e))
            hcat = None
    if hcat is None:
        h = xg
        hs = []
        for l in range(4):
            h = np.tanh(np.einsum("gds,gsf->gdf", A, h) @ Ws[l] + bs[l])
            hs.append(h)
        hcat = np.concatenate(hs, axis=-1)
    return _host_tail(hcat, inputs)
